# revision 14
# baseline (speedup 1.0000x reference)
"""Causal multi-head attention (B=1, S=4096, D=1024, H=16, HD=64) on 8 TRN2
NeuronCores.

Sharding: tensor-parallel over heads. Core c owns heads [2c, 2c+1]: Wq/Wk/Wv
column slices (128 cols) and Wo row slice (128 rows). Each core computes its
partial output projection over the full sequence in bf16; the host sums the 8
partials and adds bo (the row-parallel all-reduce done at gather time).

v3 design notes (evolution of the v1 tensor-parallel kernel):
  - The attention j-loop is ACT(exp)-bound: exp of [128, 1024] costs
    (N+352)/1.2 ns ~= 1.15us per 128-row sk tile, and 144 tiles/core is
    ~155us of ACT time. Everything else is packed underneath it.
  - attn@V is mixed-precision: OFF-DIAGONAL sk tiles (the bulk) run as
    fp8 DoubleRow matmuls over PAIRS of sk tiles (K=256/instr, V in
    e4m3, softmax weights in e5m2 straight from ACT exp), halving PE
    time; the 4 DIAGONAL tiles of each chunk run per-j in bf16, because
    the dominant self-attention weights live there and fp8 noise on
    them doesn't average out (off-diagonal weights are small relative
    to the row total, so their fp8 noise is harmless). The softmax
    denominator rides along as a ones column (row 64 of the
    accumulator) in both paths.
  - causal diagonal tiles are shrunk: scores matmuls, exp, and attn@V
    only cover valid columns; masking is a post-exp multiplicative triu
    0/1 on just the 128x128 diagonal band of the bf16 weights. Scores
    are shifted by -3 before exp (exp bias) so the off-diagonal weights
    fit fp8e4m3 range; the shift cancels in the softmax ratio.
  - QKV projections + V transposes for chunk c+1 are emitted as filler
    units inside chunk c's attention loop (interleaved with the
    deferred output-projection units), so the PE absorbs them in the
    slack under exp and ACT never waits at chunk boundaries. x^T
    activations are prefetched two chunks ahead.
  - V^T -> V via bf16 PE transposes; output partials written bf16.
"""
import sys

sys.path.insert(0, "/opt/trn_rl_repo")

import numpy as np

import concourse.bacc as bacc
import concourse.mybir as mybir
import concourse.tile as tile
from concourse.bass_utils import run_bass_kernel_spmd
from concourse.dve_ops import RECIPROCAL_APPROX_FAST, RECIP_APPROX_FAST_CONSTS

F32 = mybir.dt.float32
BF16 = mybir.dt.bfloat16
FP8 = mybir.dt.float8e4
FP8E5 = mybir.dt.float8e5
CDT = BF16                      # bf16 compute dtype for projections/scores
EXP = mybir.ActivationFunctionType.Exp
DR = mybir.MatmulPerfMode.DoubleRow
USE_FP8_AV = True               # fp8 DoubleRow for off-diagonal attn@V pairs

S, D, H, HD = 4096, 1024, 16, 64
NCORES = 8
CPC = D // NCORES  # 128 head-dim columns per core (2 heads)
NS = S // 512      # 8 chunks of 512 along the sequence
ND = D // 128      # 8 contraction chunks for the projections
NJ = S // 128      # 32 sk tiles


def _build_nc():
    nc = bacc.Bacc("TRN2", target_bir_lowering=False, debug=False,
                   num_devices=NCORES)
    xT = nc.dram_tensor("xT", [D, S], CDT, kind="ExternalInput").ap()
    w3 = nc.dram_tensor("w3", [128, 3, ND, 128], CDT, kind="ExternalInput").ap()
    wo = nc.dram_tensor("wo", [128, D], CDT, kind="ExternalInput").ap()
    bq = nc.dram_tensor("bq", [CPC, 1], F32, kind="ExternalInput").ap()
    bk = nc.dram_tensor("bk", [CPC, 1], F32, kind="ExternalInput").ap()
    bv = nc.dram_tensor("bv", [CPC, 1], F32, kind="ExternalInput").ap()
    tri2 = nc.dram_tensor("tri2", [128, 2, 128], CDT, kind="ExternalInput").ap()
    ident = nc.dram_tensor("ident", [128, 128], CDT, kind="ExternalInput").ap()
    out = nc.dram_tensor("out", [S, D], CDT, kind="ExternalOutput").ap()

    with tile.TileContext(nc) as tc:
        _emit(nc, tc, xT, w3, wo, bq, bk, bv, tri2, ident, out)
    nc.compile()
    return nc


def _emit(nc, tc, xT, w3, wo, bq, bk, bv, tri2, ident, out):
    from collections import deque
    from contextlib import ExitStack
    ctx = ExitStack()
    with ctx:
        consts = ctx.enter_context(tc.tile_pool(name="consts", bufs=1))
        persist = ctx.enter_context(tc.tile_pool(name="persist", bufs=1))
        xt_pool = ctx.enter_context(tc.tile_pool(name="xt", bufs=4))
        vt_pool = ctx.enter_context(tc.tile_pool(name="vt", bufs=2))
        pt_pool = ctx.enter_context(tc.tile_pool(name="pt", bufs=6))
        rec_pool = ctx.enter_context(tc.tile_pool(name="rec", bufs=4))
        bc_pool = ctx.enter_context(tc.tile_pool(name="bc", bufs=4))
        ost_pool = ctx.enter_context(tc.tile_pool(name="ost", bufs=8))
        # PSUM budget (16KB/partition): mm 2x2KB + sc 2x4KB + ot 2x2KB = 16KB
        ps_mm = ctx.enter_context(tc.tile_pool(name="psmm", bufs=2, space="PSUM"))
        ps_sc = ctx.enter_context(tc.tile_pool(name="pssc", bufs=2, space="PSUM"))
        ps_ot = ctx.enter_context(tc.tile_pool(name="psot", bufs=2, space="PSUM"))

        # ---- constants (wq slice first so the first matmul starts early) ---
        w_sb = consts.tile([128, 3, ND, 128], CDT)
        nc.sync.dma_start(out=w_sb[:, 0], in_=w3[:, 0])
        bq_sb = consts.tile([CPC, 1], F32)
        bk_sb = consts.tile([CPC, 1], F32)
        bv_sb = consts.tile([CPC, 1], F32)
        nc.sync.dma_start(out=bq_sb, in_=bq)
        nc.sync.dma_start(out=bk_sb, in_=bk)
        nc.sync.dma_start(out=bv_sb, in_=bv)
        id_sb = consts.tile([128, 128], CDT)
        nb_sb = consts.tile([128, 1], F32)   # exp bias: shift scores by -3 so
        nc.vector.memset(nb_sb, -3.0)        # exp fits fp8e4m3 (cancels in the
                                             # softmax ratio via the ones-col den)
        tri_sb = consts.tile([128, 2, 128], CDT)
        wo_sb = consts.tile([128, D], CDT)

        def emit_late_consts():
            nc.sync.dma_start(out=w_sb[:, 1], in_=w3[:, 1])
            nc.sync.dma_start(out=w_sb[:, 2], in_=w3[:, 2])
            nc.sync.dma_start(out=tri_sb, in_=tri2)
            nc.sync.dma_start(out=id_sb, in_=ident)
            nc.sync.dma_start(out=wo_sb, in_=wo)

        # ---- persistent activations -----------------------------------
        QT = persist.tile([128, S], CDT)      # [c(2 heads x 64), s]
        KT = persist.tile([128, S], CDT)
        # V natural + ones col, bf16 (diagonal per-j use): [p, head, j, 65]
        VPH = persist.tile([128, 2, NJ, 65], CDT)
        # fp8 copy for off-diagonal DoubleRow pairs: [p, member, jp, head, 65]
        VP8 = persist.tile([128, 2, NJ // 2, 2, 65], FP8)
        OT = persist.tile([128, S], CDT)      # rescaled attn out^T, both heads

        filler_q = deque()  # QKV units for chunk c+1 (hard deadline: popped first)
        filler_p = deque()  # deferred output-projection units (no deadline)

        xT_k = xT.rearrange("(k p) s -> p k s", p=128)
        xts = {}

        def load_xt(c):
            xt = xt_pool.tile([128, ND, 512], CDT, tag="xt", name="xt")
            cc0 = slice(512 * c, 512 * (c + 1))
            if c == 0:
                # split the first load so the first matmul starts early
                nc.sync.dma_start(out=xt[:, 0:2], in_=xT_k[:, 0:2, cc0])
                nc.sync.dma_start(out=xt[:, 2:ND], in_=xT_k[:, 2:ND, cc0])
            else:
                nc.sync.dma_start(out=xt, in_=xT_k[:, :, cc0])
            return [xt[:, k] for k in range(ND)]

        def make_qkv_units(c):
            # QKV projections + V transposes for chunk c, split into ~0.4-0.9us
            # PE units usable as fillers. Each psum-allocating unit emits its
            # consumer before two more "mm" allocations occur (ring safety).
            cc = slice(512 * c, 512 * (c + 1))
            st = {}
            units = []

            def mk_quarter(i, kq):
                def f():
                    if kq == 0:
                        st[i] = ps_mm.tile([128, 512], F32, tag="mm", name="qkvps")
                    p_ps = st[i]
                    for k in range(2 * kq, 2 * kq + 2):
                        nc.tensor.matmul(p_ps, w_sb[:, i, k], xts[c][k],
                                         start=(k == 0), stop=(k == ND - 1))
                    if kq == 3:
                        if i == 0:
                            dst, b_sb = QT[:, cc], bq_sb
                        elif i == 1:
                            dst, b_sb = KT[:, cc], bk_sb
                        else:
                            st["vt"] = vt_pool.tile([128, 512], CDT, tag="vt",
                                                    name="vt")
                            dst, b_sb = st["vt"], bv_sb
                        nc.vector.tensor_scalar_add(dst, p_ps, b_sb)
                return f

            for i in range(3):
                for kq in range(4):
                    units.append(mk_quarter(i, kq))

            def mk_tr(t):
                def f():
                    j = 4 * c + t
                    tr = ps_mm.tile([128, 128], CDT, tag="mm", name="trps")
                    nc.tensor.transpose(tr, st["vt"][:, 128 * t:128 * (t + 1)],
                                        id_sb)
                    tr3 = tr.rearrange("p (h d) -> p h d", h=2)
                    nc.vector.tensor_copy(VPH[:, :, j, 0:64], tr3)
                    # ones column (exact in bf16/fp8): in*0 + 1
                    nc.vector.tensor_scalar(VPH[:, :, j, 64:65],
                                            tr3[:, :, 0:1], 0.0, 1.0,
                                            mybir.AluOpType.mult,
                                            mybir.AluOpType.add)
                    if USE_FP8_AV:
                        nc.vector.tensor_copy(VP8[:, j % 2, j // 2, :, 0:64],
                                              tr3)
                        nc.vector.tensor_scalar(VP8[:, j % 2, j // 2, :, 64:65],
                                                tr3[:, :, 0:1], 0.0, 1.0,
                                                mybir.AluOpType.mult,
                                                mybir.AluOpType.add)
                return f

            for t in range(4):
                units.append(mk_tr(t))
            return units

        def push_proj(c):
            # output projection for s-chunk c, 8 filler units
            for t in range(4):
                for n in range(2):
                    def f(c=c, t=t, n=n):
                        ss = slice(128 * (4 * c + t), 128 * (4 * c + t + 1))
                        nn = slice(512 * n, 512 * (n + 1))
                        pr_ps = ps_mm.tile([128, 512], F32, tag="mm", name="prps")
                        nc.tensor.matmul(pr_ps, OT[:, ss], wo_sb[:, nn],
                                         start=True, stop=True)
                        o_sb = ost_pool.tile([128, 512], CDT, tag="ost",
                                             name="osb")
                        nc.vector.tensor_copy(o_sb, pr_ps)
                        nc.sync.dma_start(out=out[ss, nn], in_=o_sb)
                    filler_p.append(f)

        def pop_filler(n=1):
            for _ in range(n):
                if filler_q:
                    filler_q.popleft()()
                elif filler_p:
                    filler_p.popleft()()

        # ---- bootstrap ------------------------------------------------
        xts[0] = load_xt(0)
        emit_late_consts()
        xts[1] = load_xt(1)
        for u in make_qkv_units(0):
            u()

        for c in range(NS):
            cc = slice(512 * c, 512 * (c + 1))
            if c + 2 < NS:
                xts[c + 2] = load_xt(c + 2)
            if c + 1 < NS:
                filler_q.extend(make_qkv_units(c + 1))

            njt = 4 * (c + 1)
            ot0 = ps_ot.tile([128, 512], F32, tag="ot", name="ot0")
            ot1 = ps_ot.tile([128, 512], F32, tag="ot", name="ot1")

            def emit_av_pair(jp, ptp, _off, ot0=ot0, ot1=ot1):
                # off-diagonal pair: full width, K=256 fp8 DoubleRow
                st_ = (jp == 0)
                p4 = ptp.rearrange("p m (h q) -> p m h q", h=2)
                nc.tensor.matmul(ot0[0:65, :], VP8[:, :, jp, 0, :],
                                 p4[:, :, 0, :], start=st_, stop=False,
                                 perf_mode=DR)
                nc.tensor.matmul(ot1[0:65, :], VP8[:, :, jp, 1, :],
                                 p4[:, :, 1, :], start=st_, stop=False,
                                 perf_mode=DR)

            def emit_av_diag(j, ptd, off, c=c, njt=njt, ot0=ot0, ot1=ot1):
                st_ = (j == 0)          # only chunk 0 starts on a diagonal
                sp = (j == njt - 1)
                pt3 = ptd.rearrange("p (h q) -> p h q", h=2)
                nc.tensor.matmul(ot0[0:65, off:512], VPH[:, 0, j],
                                 pt3[:, 0, off:512], start=st_, stop=sp)
                nc.tensor.matmul(ot1[0:65, off:512], VPH[:, 1, j],
                                 pt3[:, 1, off:512], start=st_, stop=sp)

            def emit_av_single(j, pt2, off, njt=njt, ot0=ot0, ot1=ot1):
                # bf16 fallback path (USE_FP8_AV=False): per-j for all tiles
                st_, sp = (j == 0), (j == njt - 1)
                pt3 = pt2.rearrange("p (h q) -> p h q", h=2)
                nc.tensor.matmul(ot0[0:65, off:512], VPH[:, 0, j],
                                 pt3[:, 0, off:512], start=st_, stop=sp)
                nc.tensor.matmul(ot1[0:65, off:512], VPH[:, 1, j],
                                 pt3[:, 1, off:512], start=st_, stop=sp)

            pending = []
            ptp_cur = None
            for j in range(njt):
                jj = slice(128 * j, 128 * (j + 1))
                t = j - 4 * c
                off = 128 * t if t >= 0 else 0
                sc = ps_sc.tile([128, 1024], F32, tag="sc", name="sc")
                sc3 = sc.rearrange("p (h q) -> p h q", h=2)
                nc.tensor.matmul(sc3[:, 0, off:512], KT[0:64, jj],
                                 QT[0:64, 512 * c + off:512 * (c + 1)],
                                 start=True, stop=True)
                nc.tensor.matmul(sc3[:, 1, off:512], KT[64:128, jj],
                                 QT[64:128, 512 * c + off:512 * (c + 1)],
                                 start=True, stop=True)
                if USE_FP8_AV and t < 0:
                    m = j % 2
                    if m == 0:
                        ptp_cur = pt_pool.tile([128, 2, 1024], FP8,
                                               tag="pt8", name="ptp")
                    pt3 = ptp_cur[:, m].rearrange("p (h q) -> p h q", h=2)
                    nc.scalar.activation(out=pt3[:, :, :], in_=sc3[:, :, :],
                                         func=EXP, bias=nb_sb)
                    if m == 1:
                        pending.append(("pair", j // 2, ptp_cur, 0))
                else:
                    ptd = pt_pool.tile([128, 1024], CDT, tag="ptd", name="ptd")
                    pt3 = ptd.rearrange("p (h q) -> p h q", h=2)
                    nc.scalar.activation(out=pt3[:, :, off:512],
                                         in_=sc3[:, :, off:512], func=EXP,
                                         bias=nb_sb)
                    if t >= 0:
                        # diagonal band: post-exp multiplicative triu 0/1 mask
                        # on bf16 SBUF (in-place, same pattern as v1)
                        nc.vector.tensor_mul(pt3[:, :, off:off + 128],
                                             pt3[:, :, off:off + 128], tri_sb)
                    kind = "diag" if USE_FP8_AV else "single"
                    pending.append((kind, j, ptd, off))
                while len(pending) > 2:
                    kind, a, b, o2 = pending.pop(0)
                    if kind == "pair":
                        emit_av_pair(a, b, o2)
                    elif kind == "diag":
                        emit_av_diag(a, b, o2)
                    else:
                        emit_av_single(a, b, o2)
                pop_filler(4 if c == 0 else 2 if c <= 2 else 1)
            for kind, a, b, o2 in pending:
                if kind == "pair":
                    emit_av_pair(a, b, o2)
                elif kind == "diag":
                    emit_av_diag(a, b, o2)
                else:
                    emit_av_single(a, b, o2)

            # any undrained QKV units for c+1 must run before its attention
            while filler_q:
                filler_q.popleft()()

            # softmax denominators (row 64) -> rescale OT
            recs, bcs = [], []
            for ot in (ot0, ot1):
                rec = rec_pool.tile([1, 512], F32, tag="rec", name="rec")
                nc.vector._custom_dve(RECIPROCAL_APPROX_FAST, out=rec,
                                      in0=ot[64:65, :],
                                      s0=RECIP_APPROX_FAST_CONSTS["s0"],
                                      s1=RECIP_APPROX_FAST_CONSTS["s1"],
                                      imm2=RECIP_APPROX_FAST_CONSTS["imm2"])
                recs.append(rec)
            for h in (0, 1):
                bc = bc_pool.tile([64, 512], F32, tag="bc", name="bc")
                nc.gpsimd.partition_broadcast(bc, recs[h])
                bcs.append(bc)
            for h, ot in ((0, ot0), (1, ot1)):
                nc.vector.tensor_mul(OT[64 * h:64 * (h + 1), cc], ot[0:64, :],
                                     bcs[h])
            push_proj(c)

        while filler_p:
            filler_p.popleft()()


_NC_CACHE = {}


def _get_nc():
    if "nc" not in _NC_CACHE:
        _NC_CACHE["nc"] = _build_nc()
    return _NC_CACHE["nc"]


def make_in_maps(x, Wq, bq, Wk, bk, Wv, bv, Wo, bo):
    import ml_dtypes
    cdt = ml_dtypes.bfloat16
    x = np.asarray(x, np.float32).reshape(S, D)
    xT = np.ascontiguousarray(x.T).astype(cdt)
    scale = 1.0 / np.sqrt(HD)
    # additive causal mask for the 128x128 diagonal band (transposed scores:
    # pt[sk, q] masked iff sk > q within the band)
    r = np.arange(128)
    tri2 = np.where(r[:, None] > r[None, :], 0.0, 1.0).astype(np.float32)
    tri2 = np.ascontiguousarray(
        np.broadcast_to(tri2[:, None, :], (128, 2, 128))).astype(cdt)
    ident = np.eye(128, dtype=np.float32).astype(cdt)
    in_maps = []
    for c in range(NCORES):
        cs = slice(CPC * c, CPC * (c + 1))
        # w3[p, proj, k, c2] = W[128k+p, c2] for the three projections
        w3 = np.stack([np.asarray(Wq)[:, cs] * scale,
                       np.asarray(Wk)[:, cs],
                       np.asarray(Wv)[:, cs]], axis=1)  # [D, 3, 128]
        w3 = np.ascontiguousarray(
            w3.reshape(ND, 128, 3, CPC).transpose(1, 2, 0, 3)).astype(cdt)
        in_maps.append({
            "xT": xT,
            "w3": w3,
            "wo": np.ascontiguousarray(np.asarray(Wo)[cs, :]).astype(cdt),
            "bq": np.ascontiguousarray(np.asarray(bq)[cs] * scale).reshape(CPC, 1),
            "bk": np.ascontiguousarray(np.asarray(bk)[cs]).reshape(CPC, 1),
            "bv": np.ascontiguousarray(np.asarray(bv)[cs]).reshape(CPC, 1),
            "tri2": tri2,
            "ident": ident,
        })
    return in_maps


def kernel(x, Wq, bq, Wk, bk, Wv, bv, Wo, bo, _run_kwargs=None):
    nc = _get_nc()
    in_maps = make_in_maps(x, Wq, bq, Wk, bk, Wv, bv, Wo, bo)
    res = run_bass_kernel_spmd(nc, in_maps, list(range(NCORES)),
                               **(_run_kwargs or {}))
    acc = np.zeros((S, D), np.float64)
    for c in range(NCORES):
        acc += np.asarray(res.results[c]["out"]).astype(np.float64)
    full = (acc + np.asarray(bo, np.float64)).astype(np.float32)
    if _run_kwargs is not None:
        _NC_CACHE["last_results"] = res
    return full.reshape(1, S, D)
